# revision 50
# baseline (speedup 1.0000x reference)
"""Trainium2 Bass kernel for nn_Phi2_Network (dense 2-3-300-1 MLP forward).

Math: y = xa @ W31.T + relu(xa @ W21.T) @ W32.T,  xa = [x0, x1, 1].
Using relu(t) = (t + |t|)/2 and folding signs/scales into the weights:

    y = u'.xa + sum_{j: w_j>0} |z'_j| - sum_{j: w_j<=0} |z'_j|
    z' = (0.5*|w| * W21) @ xa,   u' = W31 + 0.5 * W32 @ W21

Per 128-row batch tile (one PSUM bank, 302 cols):
  - PE:  one float32r matmul: stationary XaT [3,128], moving W' [3,302]
         -> PSUM [128, 302] = [pos |z'| inputs (hp) | u'.xa + C | neg (hn) | -C]
  - ACT: activation(Abs, accum_out) over cols [0 : hp+1]  -> accA (includes lin+C)
  - DVE: tensor_tensor_reduce(op0=abs_max vs 0, scale=-1, op1=add,
         init=accA) over cols [hp+1 : 302] -> y directly (the -C col cancels C)

Data parallel across 8 cores: 62500 rows/core, padded to 504 tiles of 128.
Host pre/post: builds xa^T [3, 64512] per core, reassembles y from [128, 504].
"""

import math
import sys
from contextlib import ExitStack

import numpy as np

sys.path.insert(0, "/opt/trn_rl_repo")

import concourse.bass as bass  # noqa: E402
from concourse import bass_utils, mybir  # noqa: E402

N_CORES = 8
B_TOTAL = 500000
R = B_TOTAL // N_CORES  # 62500 rows per core
H = 300

TILES = 504  # ceil(62500/128)=489, padded to 504 = 42*12 = 126*4
RP = TILES * 128  # 64512 padded rows per core
TILES_PER_PIECE = 42  # input DMA piece granularity
STRIPS = 4
TILES_PER_STRIP = TILES // STRIPS  # 126
NC = H + 2  # 302 psum columns
KR = 8  # lhsT rows: hi/lo split per feature recovers fp32 accuracy at
        # f32r speed: [x0h, x0l, x0h, x1h, x1l, x1h, 1, 1] against
        # weight rows [ah, ah, al, bh, bh, bl, ch, cl]

F32 = mybir.dt.float32
F32R = mybir.dt.float32r

_cache: dict = {}

# test hooks (harness calls kernel() with defaults; test.py may flip these)
TRACE = False
last_results = None


N_XBUF = 3  # input piece double/triple buffering
N_PSUM = 8  # PSUM bank ring depth
N_SC = 3  # relu'd SBUF scratch ring depth
NT = 56  # PSUM tail columns reduced directly by DVE at 1x (rest go
         # through the ACT relu-copy + 2x SBUF reduce path); must stay
         # within the sign-pure [neg | +C] tail: NT <= hn + 1
PIECES = TILES // TILES_PER_PIECE  # 12
PIECE_COLS = TILES_PER_PIECE * 128  # 5376


def _build_program(hp: int, hn: int, reps: int = 1) -> bass.Bass:
    """Raw-bass pipeline with manual semaphores (v4).

    Streams:
      SP : all DMAs (w, 12 x-pieces, 4 y-strips)
      PE : 1 matmul/tile into PSUM bank t%8 (one [128,4096] tensor),
           inc sem_pe
      ACT: one Relu-copy per GROUP of 4 tiles: strided PSUM read
           [[512,4],[1,302]] -> sc[g%2] SBUF; amortizes the 222-cycle
           SBUF access across 4 tiles. inc sem_act (1/group)
      DVE: per tile, two 2x-mode SBUF reduces of the relu'd copy:
           [0:hp+1] -> accp (pos + u.xa + C), [hp+1:302] -> accn (neg + C);
           per strip one subtract: y = accp - accn. DVE never touches PSUM,
           so PE bank reuse gates only on sem_act.
    """
    nc = bass.Bass("TRN2", target_bir_lowering=False, debug=False)
    xin = nc.dram_tensor("x_lhst", [KR, RP], F32R, kind="ExternalInput").ap()
    win = nc.dram_tensor("w_rhs", [KR, NC], F32R, kind="ExternalInput").ap()
    yout = nc.dram_tensor("y_out", [128, TILES], F32, kind="ExternalOutput").ap()

    GRP = 4  # tiles per ACT copy instruction
    n_tiles = reps * TILES  # reps>1 reprocesses the same input (timing)
    n_groups = n_tiles // GRP

    with ExitStack() as ctx:
        e = ctx.enter_context
        wmat = e(nc.sbuf_tensor("wmat", [KR, NC], F32R)).ap()
        xp = [
            e(nc.sbuf_tensor(f"xp{b}", [KR, PIECE_COLS], F32R)).ap()
            for b in range(N_XBUF)
        ]
        N_SCR = 4  # sc ring depth; >=3 keeps ACT from stalling on DVE
        sc = [
            e(nc.sbuf_tensor(f"sc{b}", [128, GRP * NC], F32)).ap()
            for b in range(N_SCR)
        ]
        accp = [
            e(nc.sbuf_tensor(f"accp{b}", [128, TILES_PER_STRIP], F32)).ap()
            for b in range(2)
        ]
        accn = [
            e(nc.sbuf_tensor(f"accn{b}", [128, TILES_PER_STRIP], F32)).ap()
            for b in range(2)
        ]
        ystr = [
            e(nc.sbuf_tensor(f"ystr{b}", [128, TILES_PER_STRIP], F32)).ap()
            for b in range(2)
        ]
        ps_all = e(nc.psum_tensor("ps_all", [128, 8 * 512], F32)).ap()

        sem_pe = e(nc.semaphore("sem_pe"))
        sem_act = e(nc.semaphore("sem_act"))
        sem_dve = e(nc.semaphore("sem_dve"))
        sem_cmb = e(nc.semaphore("sem_cmb"))
        dma_w = e(nc.semaphore("dma_w"))
        dma_x = [e(nc.semaphore(f"dma_x{b}")) for b in range(N_XBUF)]
        dma_y = [e(nc.semaphore(f"dma_y{b}")) for b in range(2)]

        block = e(nc.Block())

        @block.sync
        def _(sync):
            # One serial SP stream: order DMA issues by the pipeline progress
            # (in PE-tile units) their waits require. Ties: y-stores first.
            events = []
            for p in range(reps * PIECES):
                thr = 0 if p < N_XBUF else TILES_PER_PIECE * (p - N_XBUF + 1)
                events.append((thr, 0, "x", p))
            for k in range(reps * STRIPS):
                events.append((TILES_PER_STRIP * (k + 1), -1, "y", k))
            events.sort()

            sync.dma_start(wmat, win).then_inc(dma_w, 16)
            for thr, _, kind, idx in events:
                if kind == "x":
                    if idx >= N_XBUF:
                        sync.wait_ge(sem_pe, thr)
                    pd = idx % PIECES
                    sync.dma_start(
                        xp[idx % N_XBUF],
                        xin[:, pd * PIECE_COLS : (pd + 1) * PIECE_COLS],
                    ).then_inc(dma_x[idx % N_XBUF], 16)
                else:
                    sync.wait_ge(sem_cmb, idx + 1)
                    kd = idx % STRIPS
                    sync.dma_start(
                        yout[:, kd * TILES_PER_STRIP : (kd + 1) * TILES_PER_STRIP],
                        ystr[idx % 2],
                    ).then_inc(dma_y[idx % 2], 16)
            for b in range(2):
                n_b = (reps * STRIPS - b + 1) // 2
                sync.wait_ge(dma_y[b], 16 * n_b)

        @block.tensor
        def _(pe):
            pe.wait_ge(dma_w, 16)
            for t in range(n_tiles):
                if t % TILES_PER_PIECE == 0:
                    p = t // TILES_PER_PIECE
                    pe.wait_ge(dma_x[p % N_XBUF], 16 * (p // N_XBUF + 1))
                if t >= 2 * GRP and t % GRP == 0:
                    # banks t..t+3 reusable once the ACT copy of the group
                    # two groups back has read them (DVE reads SBUF only)
                    pe.wait_ge(sem_act, t // GRP - 1)
                j = t % TILES_PER_PIECE
                lhst = xp[(t // TILES_PER_PIECE) % N_XBUF][
                    :, j * 128 : (j + 1) * 128
                ]
                bank = t % N_PSUM
                nc.tensor.matmul(
                    ps_all[:, bank * 512 : bank * 512 + NC],
                    lhst,
                    wmat,
                    start=True,
                    stop=True,
                ).then_inc(sem_pe)

        @block.scalar
        def _(act):
            for g in range(n_groups):
                base = (g % 2) * GRP * 512  # bank 0 or bank 4 origin
                src = ps_all[:, base : base + GRP * 512].rearrange(
                    "p (b w) -> p b w", w=512
                )[:, :, 0:NC]
                ins = nc.scalar.activation(
                    sc[g % N_SCR],
                    src,
                    mybir.ActivationFunctionType.Relu,
                )._wait_ge(sem_pe, GRP * (g + 1)).then_inc(sem_act)
                if g >= N_SCR:
                    # sc ring reuse: DVE reduces of group g-N_SCR all done
                    # (second embedded wait; bass asserts 1 wait by default
                    # but ACT instructions accept 2 sync waits)
                    ins.wait_op(
                        sem_dve, 2 * GRP * (g - N_SCR + 1), "sem-ge", check=False
                    )

        @block.vector
        def _(dve):
            for t in range(n_tiles):
                i = t % TILES_PER_STRIP
                s = t // TILES_PER_STRIP
                g = t // GRP
                o = (t % GRP) * NC
                if i == 0 and s >= 2:
                    # acc strip buffer reuse: combine of strip s-2 done
                    dve.wait_ge(sem_cmb, s - 1)
                # R2: 2x-mode reduce of relu'd pos + linC columns
                r2 = nc.vector.tensor_scalar(
                    out=sc[g % N_SCR][:, o : o + hp + 1],
                    in0=sc[g % N_SCR][:, o : o + hp + 1],
                    scalar1=1.0,
                    scalar2=None,
                    op0=mybir.AluOpType.mult,
                    op1=mybir.AluOpType.add,
                    accum_out=accp[s % 2][:, i : i + 1],
                ).then_inc(sem_dve)
                if t % GRP == 0:
                    r2._wait_ge(sem_act, g + 1)
                # R3: 2x-mode reduce of relu'd neg + C columns
                nc.vector.tensor_scalar(
                    out=sc[g % N_SCR][:, o + hp + 1 : o + NC],
                    in0=sc[g % N_SCR][:, o + hp + 1 : o + NC],
                    scalar1=1.0,
                    scalar2=None,
                    op0=mybir.AluOpType.mult,
                    op1=mybir.AluOpType.add,
                    accum_out=accn[s % 2][:, i : i + 1],
                ).then_inc(sem_dve)
                if i == TILES_PER_STRIP - 1:
                    # strip finished on both accumulators -> combine
                    dve.wait_ge(sem_dve, 2 * (t + 1))
                    if s >= 2:
                        # ystr buffer reuse: strip s-2 store landed
                        dve.wait_ge(dma_y[s % 2], 16 * ((s - 2) // 2 + 1))
                    nc.vector.tensor_tensor(
                        ystr[s % 2],
                        accp[s % 2],
                        accn[s % 2],
                        mybir.AluOpType.subtract,
                    ).then_inc(sem_cmb)

    return nc


def _rne11(v: np.ndarray) -> np.ndarray:
    """Round fp32 to 11 explicit mantissa bits, round-to-nearest-even —
    the exact rounding the PE applies to float32r matmul operands
    (measured on HW: 100% match on both stationary and moving sides)."""
    u = np.ascontiguousarray(v, np.float32).view(np.uint32)
    frac = u & np.uint32(0xFFF)
    base = u & np.uint32(0xFFFFF000)
    lsb = (u >> np.uint32(12)) & np.uint32(1)
    roundup = (frac > 0x800) | ((frac == 0x800) & (lsb == 1))
    out = base + roundup.astype(np.uint32) * np.uint32(0x1000)
    return out.view(np.float32).copy()


def _prepare_weights(varphi1, varphi2, l1_diag, l2_diag, l3_diag):
    v1 = np.asarray(varphi1, np.float64)
    v2 = np.asarray(varphi2, np.float64)
    l1 = np.asarray(l1_diag, np.float64)
    l2 = np.asarray(l2_diag, np.float64)
    l3 = np.asarray(l3_diag, np.float64)

    w21 = v1 * l1[None, :] - l2[:, None] * v1  # [300, 3]
    w32 = (v2 * l2[None, :] - l3[:, None] * v2)[0]  # [300]
    w31 = ((l3[:, None] * v2 - v2 * l2[None, :]) @ v1)[0]  # [3]

    wp = np.abs(w32)[:, None] * w21  # [300, 3]

    pos = np.where(w32 > 0)[0]
    neg = np.where(w32 <= 0)[0]
    hp, hn = len(pos), len(neg)

    # shift constant keeping the linear column strictly positive (|x| < 16)
    bound = (abs(w31[0]) + abs(w31[1])) * 16.0 + abs(w31[2])
    c_shift = float(2.0 ** math.ceil(math.log2(max(2.0 * bound, 8.0))))

    w3 = np.zeros((3, NC), np.float64)
    w3[:, :hp] = wp[pos].T
    w3[:, hp] = [w31[0], w31[1], w31[2] + c_shift]
    w3[:, hp + 1 : hp + 1 + hn] = wp[neg].T
    w3[:, NC - 1] = [0.0, 0.0, c_shift]

    w3f = w3.astype(np.float32)
    whi = _rne11(w3f)
    wlo = w3f - whi  # exact in fp32
    wmat = np.zeros((KR, NC), np.float32)
    wmat[0] = whi[0]
    wmat[1] = whi[0]
    wmat[2] = wlo[0]
    wmat[3] = whi[1]
    wmat[4] = whi[1]
    wmat[5] = wlo[1]
    wmat[6] = whi[2]
    wmat[7] = wlo[2]
    return wmat, hp, hn


def _build_xat(xs: np.ndarray, rp: int) -> np.ndarray:
    """Feature-major hi/lo-split input block [KR, rp] for n rows of x."""
    n = xs.shape[0]
    xat = np.zeros((KR, rp), np.float32)
    x0h = _rne11(xs[:, 0])
    x1h = _rne11(xs[:, 1])
    xat[0, :n] = x0h
    xat[1, :n] = xs[:, 0] - x0h
    xat[2, :n] = x0h
    xat[3, :n] = x1h
    xat[4, :n] = xs[:, 1] - x1h
    xat[5, :n] = x1h
    xat[6, :n] = 1.0
    xat[7, :n] = 1.0
    return xat


def kernel(x, varphi1, varphi2, l1_diag, l2_diag, l3_diag):
    x = np.ascontiguousarray(np.asarray(x, np.float32))
    assert x.shape == (B_TOTAL, 2), x.shape

    wmat, hp, hn = _prepare_weights(varphi1, varphi2, l1_diag, l2_diag, l3_diag)

    key = (hp, hn)
    if key not in _cache:
        _cache[key] = _build_program(hp, hn)
    nc = _cache[key]

    in_maps = []
    for c in range(N_CORES):
        xs = x[c * R : (c + 1) * R]
        in_maps.append({"x_lhst": _build_xat(xs, RP), "w_rhs": wmat})

    kwargs = {}
    if TRACE:
        kwargs = dict(trace=True, trace_cores=[0])
    res = bass_utils.run_bass_kernel_spmd(
        nc, in_maps, core_ids=list(range(N_CORES)), **kwargs
    )
    global last_results
    last_results = res

    ys = []
    for c in range(N_CORES):
        out = np.asarray(res.results[c]["y_out"])  # [128, TILES]
        ys.append(out.T.reshape(-1)[:R])
    return np.concatenate(ys).reshape(B_TOTAL, 1).astype(np.float32)


if __name__ == "__main__":
    rng = np.random.default_rng(0)
    demo = {
        "x": rng.standard_normal((B_TOTAL, 2), dtype=np.float32),
        "varphi1": (0.1 * rng.standard_normal((300, 3))).astype(np.float32),
        "varphi2": (0.1 * rng.standard_normal((1, 300))).astype(np.float32),
        "l1_diag": rng.standard_normal(3).astype(np.float32),
        "l2_diag": rng.standard_normal(300).astype(np.float32),
        "l3_diag": rng.standard_normal(1).astype(np.float32),
    }
    y = kernel(**demo)
    print("kernel output", y.shape, y[:4, 0])
